# revision 14
# baseline (speedup 1.0000x reference)
"""Multi-LoRA batched GEMM on 8 Trainium2 NeuronCores (SPMD, data-parallel).

Problem:
    y[b, s, r] = sum_h x[b, s, h] * weight[adapter_ids[b], r, h]
    x: [32, 2048, 4096] f32, weight: [16, 64, 4096] f32, adapter_ids: [32] int
    y: [32, 2048, 64] f32

Strategy:
  - Host: gather per-sample weights weight[adapter_ids] (tiny), shard the
    batch 4 samples per core (data parallel, no collectives), and split x / w
    into exact bf16 hi+lo pairs (x == hi + lo to ~16 mantissa bits).
  - Device (per core): for each (sample, s-chunk), one xbar DMA-transpose
    pulls x[i, chunk, :] into SBUF as [128 (h%128), H/128 (h//128), s_chunk]
    with fully contiguous 8KB row reads from HBM.  Then 3 * H/128 accumulating
    bf16 matmuls per chunk (hi*hi + hi*lo + lo*hi; the lo*lo term is ~2^-32
    relative and dropped) into a [64, s_chunk] PSUM bank -> fp32-grade result.
  - y is produced transposed ([64, s] per sample), staged in SBUF, stored with
    one DMA at the end; the host transposes it back (tiny, 16 MB total).
"""

import numpy as np

B, S, H, R = 32, 2048, 4096, 64
N_CORES = 8
B_LOC = B // N_CORES

# Tunables for the device program.
S_CHUNK = 256          # tokens per transposed x tile / matmul moving dim
SPLIT = True           # bf16 hi+lo split (3 matmul passes) vs plain bf16 (1)
X_BUFS = 3             # buffering for the transposed x tiles

_prog_cache = {}


def _build_program(b_loc, s, h, r, s_chunk, split, x_bufs=X_BUFS):
    """Build the SPMD Bass/Tile program for one NeuronCore."""
    from contextlib import ExitStack

    import concourse.bass as bass
    import concourse.mybir as mybir
    import concourse.tile as tile

    K = h // 128               # h-tiles
    C = s // s_chunk           # s-chunks per sample
    bf16 = mybir.dt.bfloat16
    f32 = mybir.dt.float32

    nc = bass.Bass()

    n_half = 2 if split else 1
    x_names = ["x_hi"] + (["x_lo"] if split else [])
    x_dram = [
        nc.declare_dram_parameter(n, [b_loc, s, h], bf16, isOutput=False)
        for n in x_names
    ]
    # hi and lo weight tables stacked in one tensor -> one load DMA
    w_dram = nc.declare_dram_parameter(
        "w", [128, n_half, b_loc, K, r], bf16, isOutput=False
    )
    yT = nc.declare_dram_parameter("yT", [b_loc, r, s], f32, isOutput=True)

    with tile.TileContext(nc) as tc:
        with ExitStack() as ctx:
            wpool = ctx.enter_context(tc.tile_pool(name="w", bufs=1))
            xpool = ctx.enter_context(tc.tile_pool(name="x", bufs=x_bufs))
            ypool = ctx.enter_context(tc.tile_pool(name="y", bufs=1))
            pp = ctx.enter_context(tc.tile_pool(name="ps", bufs=4, space="PSUM"))

            # All per-sample weights resident for the whole kernel:
            # wt[p, t, i, k, r] = split-half t of W_i[r, 128k + p].
            # Plain DMAs ride SWDGE (gpsimd) so the HWDGE queues carry only
            # xbar transposes: mixing both on HWDGE adds cross-lane waits to
            # the XPOSE descriptors, which overflow their sync-wait slots.
            wt = wpool.tile([128, n_half, b_loc, K, r], bf16)
            nc.gpsimd.dma_start(wt[:], w_dram[:])
            wts = [wt[:, t] for t in range(n_half)]

            # y staging: y_sb[rr, i, ss] = y[i, ss, rr]
            y_sb = ypool.tile([r, b_loc, s], f32)

            # (w_idx, x_idx) matmul passes per k-tile
            terms = [(0, 0), (0, 1), (1, 0)] if split else [(0, 0)]
            n_mm = K * len(terms)

            for i in range(b_loc):
                for c in range(C):
                    sl = slice(c * s_chunk, (c + 1) * s_chunk)
                    xts = []
                    for t, xd in enumerate(x_dram):
                        # xt[p, k, ss] = x[i, c*s_chunk + ss, 128k + p]
                        xt = xpool.tile([128, K, s_chunk], bf16, tag=f"x{t}")
                        nc.sync.dma_start(xt[:], xd[i, sl, :], transpose=True)
                        xts.append(xt)
                    pt = pp.tile([r, s_chunk], f32, tag="ps")
                    mm = 0
                    for k in range(K):
                        for (wi, xi) in terms:
                            nc.tensor.matmul(
                                pt[:, :],
                                wts[wi][:, i, k, :],
                                xts[xi][:, k, :],
                                start=(mm == 0),
                                stop=(mm == n_mm - 1),
                            )
                            mm += 1
                    # Pinned to ScalarE so the final y-store only ever needs
                    # a single ACT wait (see _fixup_sync_waits).
                    nc.scalar.copy(out=y_sb[:, i, sl], in_=pt[:, :])

            # One store at the end: yT[i, rr, ss] <- y_sb[rr, i, ss]
            nc.gpsimd.dma_start(yT.rearrange("i p s -> p i s"), y_sb[:])

    _fixup_sync_waits(nc)
    return nc


def _fixup_sync_waits(nc):
    """Walrus allows only ONE sync-wait slot on DMA_DIRECT2D_XPOSE and
    S3D3_MM hardware descriptor structs; Tile sometimes emits two.

    - DmaTransposeAnt with {PE, DMAHW} waits: on x-tile slot reuse Tile emits
      a PE wait (WAR: matmuls that read the slot's old contents) and a DMAHW
      wait (WAW: the transpose that wrote them).  The WAW is transitively
      implied by the WAR (those matmuls waited on that DMA), so drop it.
    - Matmult with 2 waits: move one wait onto the immediately-preceding
      Ldweights on the PE queue.  The NX sequencer processes waits at
      dispatch in program order, so blocking the LDW blocks the MM the same
      way — semantics unchanged.
    - The final y-store DMACopy with {ACT, DMAHW} waits: the ACT wait is the
      last PSUM->SBUF copy, which ran after all matmuls, which waited on all
      transposes — so the DMAHW wait (Tile's transpose/copy SDMA
      serialization) is transitively satisfied and no transpose can still be
      in flight.  Drop it.
    - The kernel-tail SP Drain waits on every proc's final tick (12 sems
      here, over the CTRL struct limit).  Everything in the kernel is
      upstream of the y store (store <- last ACT copy <- all matmuls <- all
      transposes + weight load), so waiting on the store's completion sem
      alone is equivalent.
    Anything not matching these exact patterns raises.
    """
    import bass_rust

    def waits_of(inst):
        si = inst.sync_info
        return list(si.on_wait) if si is not None else []

    def set_waits(inst, waits):
        si = inst.sync_info
        upd = list(si.on_update) if si is not None else []
        inst.sync_info = bass_rust.SyncInfo(on_wait=waits, on_update=upd)

    # Completion sem of the final (y-store) DMACopy — the transitive root.
    store_update = None
    for blk in nc.m.functions[0].blocks:
        for inst in blk.instructions:
            if type(inst).__name__ == "InstDMACopy":
                si = inst.sync_info
                if si is not None and si.on_update:
                    store_update = si.on_update[0]

    for blk in nc.m.functions[0].blocks:
        prev_pe = None
        for inst in blk.instructions:
            tname = type(inst).__name__
            if tname == "InstDmaTransposeAnt":
                waits = waits_of(inst)
                if len(waits) > 1:
                    pe = [w for w in waits if w.ant_name.startswith("PE")]
                    dma = [w for w in waits if w.ant_name.startswith("DMAHW")]
                    if not (len(waits) == 2 and len(pe) == 1 and len(dma) == 1):
                        raise RuntimeError(
                            f"unexpected waits on {inst.name}: "
                            f"{[(w.ant_name, w.wait_value) for w in waits]}"
                        )
                    set_waits(inst, pe)
            elif tname == "InstDrain":
                waits = waits_of(inst)
                if len(waits) > 1:
                    keep = [
                        w for w in waits
                        if store_update is not None
                        and w.ant_name == store_update.ant_name
                        and w.wait_value == store_update.update_value
                    ]
                    if len(keep) != 1:
                        raise RuntimeError(
                            f"tail drain {inst.name} lacks y-store wait: "
                            f"{[(w.ant_name, w.wait_value) for w in waits]} "
                            f"store={store_update}"
                        )
                    set_waits(inst, keep)
            elif tname == "InstDMACopy":
                waits = waits_of(inst)
                if len(waits) > 1:
                    act = [w for w in waits if w.ant_name.startswith("Activation")]
                    dma = [w for w in waits if w.ant_name.startswith("DMAHW")]
                    if not (len(waits) == 2 and len(act) == 1 and len(dma) == 1):
                        raise RuntimeError(
                            f"unexpected waits on {inst.name}: "
                            f"{[(w.ant_name, w.wait_value) for w in waits]}"
                        )
                    set_waits(inst, act)
            elif tname in ("InstMatmult", "InstLdweights"):
                waits = waits_of(inst)
                if tname == "InstMatmult" and len(waits) > 1:
                    if (
                        prev_pe is None
                        or type(prev_pe).__name__ != "InstLdweights"
                        or len(waits_of(prev_pe)) != 0
                        or len(waits) != 2
                    ):
                        raise RuntimeError(
                            f"cannot redistribute waits on {inst.name}: "
                            f"{[(w.ant_name, w.wait_value) for w in waits]}"
                        )
                    set_waits(prev_pe, [waits[0]])
                    set_waits(inst, [waits[1]])
                prev_pe = inst


def _get_program(b_loc=B_LOC, s=S, h=H, r=R, s_chunk=S_CHUNK, split=SPLIT,
                 x_bufs=X_BUFS):
    key = (b_loc, s, h, r, s_chunk, split, x_bufs)
    if key not in _prog_cache:
        _prog_cache[key] = _build_program(*key)
    return _prog_cache[key]


def _split_bf16(a):
    """Exact-ish split of f32 array into bf16 hi + lo (a ~= hi + lo)."""
    import ml_dtypes

    hi = np.asarray(a, dtype=np.float32).astype(ml_dtypes.bfloat16)
    lo = (np.asarray(a, dtype=np.float32) - hi.astype(np.float32)).astype(
        ml_dtypes.bfloat16
    )
    return hi, lo


def _prep_inputs(x, adapter_ids, weight, split=SPLIT):
    """Host-side: gather weights, relayout, bf16-split, shard per core."""
    x = np.asarray(x)
    ids = np.asarray(adapter_ids).astype(np.int64)
    weight = np.asarray(weight)

    # Gather per-sample weights and relayout to wt[p, b, k, r] = W_b[r, 128k+p]
    w_g = weight[ids]                              # [B, R, H] f32
    K = H // 128
    wt = np.ascontiguousarray(
        w_g.transpose(2, 0, 1).reshape(K, 128, B, R).transpose(1, 2, 0, 3)
    )                                              # [128, B, K, R]

    x_hi, x_lo = _split_bf16(x)
    wt_hi, wt_lo = _split_bf16(wt)

    in_maps = []
    for cidx in range(N_CORES):
        bsl = slice(cidx * B_LOC, (cidx + 1) * B_LOC)
        halves = [wt_hi[:, bsl]] + ([wt_lo[:, bsl]] if split else [])
        m = {
            "x_hi": x_hi[bsl],
            "w": np.ascontiguousarray(np.stack(halves, axis=1)),
        }
        if split:
            m["x_lo"] = x_lo[bsl]
        in_maps.append(m)
    return in_maps


def _run(x, adapter_ids, weight, trace=False, s_chunk=S_CHUNK, split=SPLIT,
         x_bufs=X_BUFS):
    from concourse.bass_utils import run_bass_kernel_spmd

    nc = _get_program(s_chunk=s_chunk, split=split, x_bufs=x_bufs)
    in_maps = _prep_inputs(x, adapter_ids, weight, split=split)
    res = run_bass_kernel_spmd(nc, in_maps, list(range(N_CORES)), trace=trace)

    outs = []
    for cidx in range(N_CORES):
        yT = res.results[cidx]["yT"]               # [B_LOC, R, S] f32
        outs.append(np.transpose(yT, (0, 2, 1)))   # [B_LOC, S, R]
    y = np.concatenate(outs, axis=0).astype(np.float32)
    return y, res


def kernel(x, adapter_ids, weight):
    y, _ = _run(x, adapter_ids, weight, trace=False)
    return y
